# revision 14
# baseline (speedup 1.0000x reference)
"""Sliding-window causal self-attention (WINDOW=256) on 8 trn2 NeuronCores.

Sharding: 8 cores = 4 batch items x 2 sequence halves (1024 queries each).
Each core receives x pre-transposed with a 256-row key/value halo and
computes its output chunk fully independently (no collectives); the host
concatenates.

On-core dataflow (transpose-free, "key-major" attention):
  x_T [C=768, 1280]  --PE-->  Q_T [768, 1024], K_T [768, 1280]  (head dim on
  partitions), V [1280, 768] (+ a 64-wide ones block per head for softmax
  denominators).
  Per head, per 128-key tile: S_T [128k, 384q] = K_T.T @ Q_T (K=64 matmul),
  E = exp(S/8) via ScalarE (scale fused), band+validity mask multiply (DVE),
  then AV matmuls accumulate Y (rows 0-63) and the replicated denominator
  (rows 64-127) into a 2-bank PSUM tile holding both query halves.
  Normalize with DVE reciprocal_approx_fast + tensor_tensor, project with
  W_proj.  Biases: bk drops out of softmax exactly; bq is fused into the Q
  PSUM->SBUF copy per partition; bv/bp are folded into a host-side
  post-add of (bv @ W_proj + bp), exact because attention rows sum to 1.
"""

import numpy as np
import ml_dtypes

import concourse.bass as bass
import concourse.bacc as bacc
import concourse.mybir as mybir
from concourse.tile import TileContext
from concourse.bass_utils import run_bass_kernel_spmd

F32 = mybir.dt.float32
BF16 = mybir.dt.bfloat16
FP8 = mybir.dt.float8e4
DR = mybir.MatmulPerfMode.DoubleRow
AF = mybir.ActivationFunctionType
OP = mybir.AluOpType

N_HEAD = 12
WINDOW = 256
B, T, C = 4, 2048, 768
HD = C // N_HEAD              # 64
TQ = 1024                     # queries per core
HALO = 256
ROWS = TQ + HALO              # 1280 rows of k/v per core
NCT = C // 128                # 6 contraction tiles
NKT = ROWS // 128             # 10 key tiles
WIN = 384                     # q-window width per key tile
SCALE = 1.0 / float(np.sqrt(HD))

# q-window start per key tile (compile-time, same on every core)
QS = [min(max(128 * (kt - 2), 0), TQ - WIN) for kt in range(NKT)]


def _build_nc() -> bass.Bass:
    nc = bacc.Bacc()

    x_t = nc.dram_tensor("x_t", [C, ROWS], BF16, kind="ExternalInput")
    wq = nc.dram_tensor("wq", [C, C], BF16, kind="ExternalInput")
    wk = nc.dram_tensor("wk", [C, C], BF16, kind="ExternalInput")
    wv = nc.dram_tensor("wv", [C, C], BF16, kind="ExternalInput")
    wp = nc.dram_tensor("wp", [C, C], BF16, kind="ExternalInput")
    bq_t = nc.dram_tensor("bq_t", [128, NCT], F32, kind="ExternalInput")
    mask = nc.dram_tensor("mask", [128, NKT * WIN], BF16, kind="ExternalInput")
    out = nc.dram_tensor("out", [TQ, C], F32, kind="ExternalOutput")

    with TileContext(nc) as tc:
        with (
            tc.tile_pool(name="persist", bufs=1) as pp,
            tc.tile_pool(name="work", bufs=3) as wk_pool,
            tc.tile_pool(name="et", bufs=2) as et_pool,
            tc.tile_pool(name="psA", bufs=2, space="PSUM") as psA,
            tc.tile_pool(name="psS", bufs=2, space="PSUM") as psS,
        ):
            # ---- persistent SBUF tensors ----
            xt_sb = pp.tile([128, NCT, ROWS], BF16)
            wq_sb = pp.tile([128, NCT, C], BF16)
            wk_sb = pp.tile([128, NCT, C], BF16)
            wv_sb = pp.tile([128, NCT, C], BF16)
            wp_sb = pp.tile([128, NCT, C], BF16)
            bq_sb = pp.tile([128, NCT], F32)
            mask_sb = pp.tile([128, NKT, WIN], BF16)

            qt_sb = pp.tile([128, NCT, TQ], BF16)     # Q_T: head dims on partitions
            kt_sb = pp.tile([128, NCT, ROWS], BF16)   # K_T
            v_sb = pp.tile([128, NKT, N_HEAD, 128], BF16)  # [ones(64) | V(64)]
            yn_sb = pp.tile([128, NCT, TQ], BF16)     # normalized Y_T

            # Interleave per-c-tile wk/xt DMAs so K-proj starts ~1.5us in
            # instead of waiting for both full transfers.
            xt_r = x_t.rearrange("(t p) n -> p t n", p=128)
            wk_r = wk.rearrange("(t p) n -> p t n", p=128)
            for c in range(NCT):
                nc.sync.dma_start(wk_sb[:, c, :], wk_r[:, c, :])
                nc.sync.dma_start(xt_sb[:, c, :], xt_r[:, c, :])
            nc.sync.dma_start(wq_sb[:], wq.rearrange("(t p) n -> p t n", p=128))
            nc.sync.dma_start(bq_sb[:], bq_t[:])
            nc.sync.dma_start(wv_sb[:], wv.rearrange("(t p) n -> p t n", p=128))
            nc.sync.dma_start(wp_sb[:], wp.rearrange("(t p) n -> p t n", p=128))
            nc.sync.dma_start(mask_sb[:], mask.rearrange("p (k w) -> p k w", w=WIN))
            # ones columns for the softmax denominators: SBUF-only, on gpsimd
            nc.gpsimd.memset(v_sb[:, :, :, 0:HD], 1.0)

            # ---- phase 2: projections ----
            # K_T / Q_T: out[outdim_tile, seq] = W.T @ x_T, 2-bank PSUM tiles
            def proj_T(w_sb, dst, xcol0, dcol0, ncols, bias_m=None):
                nj = (ncols + 511) // 512
                for m in range(NCT):
                    ps = psA.tile([128, 2, 512], F32, tag="mm")
                    for c in range(NCT):
                        for j in range(nj):
                            w = min(512, ncols - j * 512)
                            nc.tensor.matmul(
                                ps[:, j, :w],
                                w_sb[:, c, m * 128:(m + 1) * 128],
                                xt_sb[:, c, xcol0 + j * 512:xcol0 + j * 512 + w],
                                start=(c == 0),
                                stop=(c == NCT - 1),
                                skip_group_check=True,
                            )
                    src = (
                        ps[:, 0, :ncols] if ncols <= 512
                        else ps[:, :, :].rearrange("p a b -> p (a b)")[:, :ncols]
                    )
                    dstv = dst[:, m, dcol0:dcol0 + ncols]
                    if bias_m is not None:
                        nc.vector.tensor_scalar_add(dstv, src, bias_m[:, m:m + 1])
                    else:
                        nc.scalar.copy(dstv, src)

            for col0, ncols in ((0, 1024), (1024, 256)):
                proj_T(wk_sb, kt_sb, col0, col0, ncols)
            proj_T(wq_sb, qt_sb, HALO, 0, 1024, bias_m=bq_sb)

            # V: out[row_tile, vcols] = x_T.T @ W_v   (one 2-group tile per r)
            for r in range(NKT):
                ps = psA.tile([128, 2, 512], F32, tag="mm")
                for c in range(NCT):
                    for j in range(2):
                        nc.tensor.matmul(
                            ps[:, j, :384],
                            xt_sb[:, c, r * 128:(r + 1) * 128],
                            wv_sb[:, c, j * 384:j * 384 + 384],
                            start=(c == 0),
                            stop=(c == NCT - 1),
                            skip_group_check=True,
                        )
                # [128, 2, 6, 64] -> v_sb[:, r, h, 64:128]
                src = ps[:, :, :384].rearrange("p a (h d) -> p a h d", d=HD)
                dst = v_sb[:, r, :, HD:128].rearrange("p (a h) d -> p a h d", a=2)
                if r % 2 == 0:
                    nc.scalar.copy(dst, src)
                else:
                    nc.vector.tensor_copy(out=dst, in_=src)

            # ---- phase 3: attention per head ----
            for h in range(N_HEAD):
                ct = h // 2
                p0 = (h % 2) * HD
                et = et_pool.tile([128, NKT, WIN], BF16, tag="et")
                for kt2 in range(0, NKT, 2):
                    ps_s = psS.tile([128, 2, 512], F32, tag="ss")
                    for j in range(2):
                        kt = kt2 + j
                        nc.tensor.matmul(
                            ps_s[:, j, :WIN],
                            kt_sb[p0:p0 + HD, ct, kt * 128:(kt + 1) * 128],
                            qt_sb[p0:p0 + HD, ct, QS[kt]:QS[kt] + WIN],
                            start=True, stop=True,
                        )
                    nc.scalar.activation(
                        et[:, kt2:kt2 + 2, :], ps_s[:, :, :WIN], AF.Exp,
                        scale=SCALE,
                    )
                    # band/validity mask; per-pair so AV can start early
                    eng = nc.gpsimd if h >= 10 else nc.vector
                    eng.tensor_tensor(
                        et[:, kt2:kt2 + 2, :],
                        et[:, kt2:kt2 + 2, :],
                        mask_sb[:, kt2:kt2 + 2, :],
                        OP.mult,
                    )

                # AV: one 2-bank tile holds both query halves (+denominators)
                ps_y = psA.tile([128, 2, 512], F32, tag="mm")
                for half in range(2):
                    qb0 = half * 4
                    mms = []
                    for kt in range(NKT):
                        for qb in (kt - 2, kt - 1, kt):
                            if qb0 <= qb < qb0 + 4:
                                mms.append((kt, qb))
                    for i, (kt, qb) in enumerate(mms):
                        j0 = qb * 128 - QS[kt]
                        nc.tensor.matmul(
                            ps_y[:, half, (qb - qb0) * 128:(qb - qb0 + 1) * 128],
                            v_sb[:, kt, h, :],
                            et[:, kt, j0:j0 + 128],
                            start=(i == 0),
                            stop=(i == len(mms) - 1),
                            skip_group_check=True,
                        )
                # normalize: D sits in rows 0:64 (ones block first), Y in
                # 64:128.  Custom-DVE ops need partition base 0; TT's PSUM
                # input base is independent of the SBUF base (HW-verified).
                rec = wk_pool.tile([HD, 2, 512], F32, tag="rec", bufs=2)
                nc.vector.reciprocal_approx_fast(
                    rec[:, :, :], ps_y[0:HD, :, :]
                )
                nc.vector.tensor_tensor(
                    yn_sb[p0:p0 + HD, ct, :].rearrange("p (a b) -> p a b", a=2),
                    ps_y[HD:128, :, :],
                    rec[:, :, :],
                    OP.mult,
                )

            # ---- phase 4: output projection ----
            for qb in range(8):
                o_sb = wk_pool.tile([128, C], F32, tag="osb")
                ps = psA.tile([128, 2, 512], F32, tag="mm")
                for c in range(NCT):
                    for j in range(2):
                        nc.tensor.matmul(
                            ps[:, j, :384],
                            yn_sb[:, c, qb * 128:(qb + 1) * 128],
                            wp_sb[:, c, j * 384:j * 384 + 384],
                            start=(c == 0),
                            stop=(c == NCT - 1),
                            skip_group_check=True,
                        )
                src = ps[:, :, :384]
                dst = o_sb[:, :].rearrange("p (a b) -> p a b", a=2)
                if qb % 2 == 0:
                    nc.scalar.copy(dst, src)
                else:
                    nc.vector.tensor_copy(out=dst, in_=src)
                nc.sync.dma_start(out[qb * 128:(qb + 1) * 128, :], o_sb[:])

    nc.compile()
    return nc


_NC_CACHE = []


def _get_nc() -> bass.Bass:
    if not _NC_CACHE:
        _NC_CACHE.append(_build_nc())
    return _NC_CACHE[0]


def _make_mask(half: int) -> np.ndarray:
    chunk_start = half * TQ
    p = np.arange(128)[:, None, None]
    kt = np.arange(NKT)[None, :, None]
    j = np.arange(WIN)[None, None, :]
    lk = 128 * kt + p
    qi = np.array(QS)[None, :, None] + j
    band = (qi >= lk - WINDOW) & (qi <= lk - 1)
    exists = (chunk_start - HALO + lk) >= 0
    m = (band & exists).astype(ml_dtypes.bfloat16)
    return m.reshape(128, NKT * WIN)


def build_in_maps(x, W_attn, b_attn, W_proj, b_proj):
    x = np.asarray(x, dtype=np.float32)
    W_attn = np.asarray(W_attn, dtype=np.float32)
    b_attn = np.asarray(b_attn, dtype=np.float32)

    bf = ml_dtypes.bfloat16
    wq_h = W_attn[:, 0:C].astype(bf)
    wk_h = W_attn[:, C:2 * C].astype(bf)
    wv_h = W_attn[:, 2 * C:3 * C].astype(bf)
    wp_h = np.asarray(W_proj, dtype=np.float32).astype(bf)
    # bq laid out [128, NCT]: partition p of tile m is outdim m*128+p
    bq_h = np.ascontiguousarray(
        b_attn[0:C].reshape(NCT, 128).T
    ).astype(np.float32)
    masks = [_make_mask(0), _make_mask(1)]

    in_maps = []
    for core in range(8):
        b, half = divmod(core, 2)
        start = half * TQ - HALO
        if start < 0:
            x_win = np.concatenate(
                [np.zeros((HALO, C), np.float32), x[b, 0:TQ]], axis=0)
        else:
            x_win = x[b, start:start + ROWS]
        x_t = np.ascontiguousarray(x_win.T).astype(bf)
        in_maps.append({
            "x_t": x_t, "wq": wq_h, "wk": wk_h, "wv": wv_h, "wp": wp_h,
            "bq_t": bq_h, "mask": masks[half],
        })
    return in_maps


def kernel(x, W_attn, b_attn, W_proj, b_proj):
    in_maps = build_in_maps(x, W_attn, b_attn, W_proj, b_proj)
    nc = _get_nc()
    res = run_bass_kernel_spmd(nc, in_maps, list(range(8)))
    b_attn = np.asarray(b_attn, dtype=np.float32)
    W_proj = np.asarray(W_proj, dtype=np.float32)
    b_proj = np.asarray(b_proj, dtype=np.float32)
    # bv passes through softmax (rows sum to 1) and then W_proj; bp direct.
    out_bias = b_attn[2 * C:3 * C] @ W_proj + b_proj
    y = np.empty((B, T, C), dtype=np.float32)
    for core in range(8):
        b, half = divmod(core, 2)
        y[b, half * TQ:(half + 1) * TQ, :] = res.results[core]["out"] + out_bias
    return y


# revision 16
# speedup vs baseline: 1.0540x; 1.0540x over previous
"""Sliding-window causal self-attention (WINDOW=256), head-split sharding.

8 cores = 4 batch items x 2 head-groups (6 heads each).  Each core runs the
full T=2048 sequence for its 6 heads (no halo, no sequence overlap) and
produces a PARTIAL output projection [T, C]; the host sums the pair of
head-group partials per batch item and adds (bv @ W_proj + bp).

Same on-core dataflow as the seq-split kernel: key-major S tiles, fused
exp on ScalarE, band-mask multiply on DVE/GpSimd, AV matmuls with a
[ones|V] stationary block so softmax denominators land in PSUM rows 0:64
(reciprocal_approx_fast needs partition base 0), DVE normalize.
"""

import numpy as np
import ml_dtypes

import concourse.bass as bass
import concourse.bacc as bacc
import concourse.mybir as mybir
from concourse.tile import TileContext
from concourse.bass_utils import run_bass_kernel_spmd

F32 = mybir.dt.float32
BF16 = mybir.dt.bfloat16
AF = mybir.ActivationFunctionType
OP = mybir.AluOpType

N_HEAD = 12
WINDOW = 256
B, T, C = 4, 2048, 768
HD = C // N_HEAD              # 64
NH = 6                        # heads per core
GD = NH * HD                  # 384 projection dims per core
NCT = C // 128                # 6 contraction tiles (x -> qkv)
NM = GD // 128                # 3 qkv outdim tiles
NKT = T // 128                # 16 key tiles
WIN = 384                     # q-window width per key tile
SCALE = 1.0 / float(np.sqrt(HD))

# q-window start per key tile: band is [lk, lk+255] (keys precede queries)
QS = [min(128 * kt, T - WIN) for kt in range(NKT)]


def _build_nc() -> bass.Bass:
    nc = bacc.Bacc()

    x_t = nc.dram_tensor("x_t", [C, T], BF16, kind="ExternalInput")
    wq = nc.dram_tensor("wq", [C, GD], BF16, kind="ExternalInput")
    wk = nc.dram_tensor("wk", [C, GD], BF16, kind="ExternalInput")
    wv = nc.dram_tensor("wv", [C, GD], BF16, kind="ExternalInput")
    wp = nc.dram_tensor("wp", [GD, C], BF16, kind="ExternalInput")
    bq_t = nc.dram_tensor("bq_t", [128, NM], F32, kind="ExternalInput")
    mask = nc.dram_tensor("mask", [128, NKT * WIN], BF16, kind="ExternalInput")
    out = nc.dram_tensor("out", [T, C], F32, kind="ExternalOutput")

    with TileContext(nc) as tc:
        with (
            tc.tile_pool(name="persist", bufs=1) as pp,
            tc.tile_pool(name="work", bufs=3) as wk_pool,
            tc.tile_pool(name="et", bufs=2) as et_pool,
            tc.tile_pool(name="psA", bufs=2, space="PSUM") as psA,
            tc.tile_pool(name="psS", bufs=2, space="PSUM") as psS,
        ):
            xt_sb = pp.tile([128, NCT, T], BF16)
            wq_sb = pp.tile([128, NCT, GD], BF16)
            wk_sb = pp.tile([128, NCT, GD], BF16)
            wv_sb = pp.tile([128, NCT, GD], BF16)
            wp_sb = pp.tile([128, NM, C], BF16)
            bq_sb = pp.tile([128, NM], F32)
            mask_sb = pp.tile([128, NKT, WIN], BF16)

            qt_sb = pp.tile([128, NM, T], BF16)
            kt_sb = pp.tile([128, NM, T], BF16)
            v_sb = pp.tile([128, NKT, NH, 128], BF16)   # [ones(64) | V(64)]
            yn_sb = pp.tile([128, NM, T], BF16)

            xt_r = x_t.rearrange("(t p) n -> p t n", p=128)
            nc.sync.dma_start(wk_sb[:], wk.rearrange("(t p) n -> p t n", p=128))
            for c in range(NCT):
                nc.sync.dma_start(xt_sb[:, c, :], xt_r[:, c, :])
            nc.sync.dma_start(wq_sb[:], wq.rearrange("(t p) n -> p t n", p=128))
            nc.sync.dma_start(bq_sb[:], bq_t[:])
            nc.sync.dma_start(wv_sb[:], wv.rearrange("(t p) n -> p t n", p=128))
            nc.sync.dma_start(wp_sb[:], wp.rearrange("(t p) n -> p t n", p=128))
            nc.sync.dma_start(mask_sb[:], mask.rearrange("p (k w) -> p k w", w=WIN))
            nc.vector.memset(v_sb[:, :, :, 0:HD], 1.0)

            # ---- projections ----
            def proj_T(w_sb, dst, bias_m=None):
                for m in range(NM):
                    for g0 in (0, 1024):
                        ps = psA.tile([128, 2, 512], F32, tag="mm")
                        for c in range(NCT):
                            for j in range(2):
                                nc.tensor.matmul(
                                    ps[:, j, :],
                                    w_sb[:, c, m * 128:(m + 1) * 128],
                                    xt_sb[:, c, g0 + j * 512:g0 + (j + 1) * 512],
                                    start=(c == 0),
                                    stop=(c == NCT - 1),
                                    skip_group_check=True,
                                )
                        src = ps[:, :, :].rearrange("p a b -> p (a b)")
                        dstv = dst[:, m, g0:g0 + 1024]
                        if bias_m is not None:
                            nc.vector.tensor_scalar_add(
                                dstv, src, bias_m[:, m:m + 1])
                        else:
                            nc.scalar.copy(dstv, src)

            proj_T(wk_sb, kt_sb)
            proj_T(wq_sb, qt_sb, bias_m=bq_sb)

            # V: two row-tiles per PSUM tile
            for r2 in range(0, NKT, 2):
                ps = psA.tile([128, 2, 512], F32, tag="mm")
                for rr in range(2):
                    for c in range(NCT):
                        nc.tensor.matmul(
                            ps[:, rr, :GD],
                            xt_sb[:, c, (r2 + rr) * 128:(r2 + rr + 1) * 128],
                            wv_sb[:, c, :],
                            start=(c == 0),
                            stop=(c == NCT - 1),
                            skip_group_check=True,
                        )
                src = ps[:, :, :GD].rearrange("p a (h d) -> p a h d", d=HD)
                dst = v_sb[:, r2:r2 + 2, :, HD:128]
                if (r2 // 2) % 2 == 0:
                    nc.scalar.copy(dst, src)
                else:
                    nc.vector.tensor_copy(out=dst, in_=src)

            # ---- attention ----
            for h in range(NH):
                ct = h // 2
                p0 = (h % 2) * HD
                et = et_pool.tile([128, NKT, WIN], BF16, tag="et")
                for kt2 in range(0, NKT, 2):
                    ps_s = psS.tile([128, 2, 512], F32, tag="ss")
                    for j in range(2):
                        kt = kt2 + j
                        nc.tensor.matmul(
                            ps_s[:, j, :WIN],
                            kt_sb[p0:p0 + HD, ct, kt * 128:(kt + 1) * 128],
                            qt_sb[p0:p0 + HD, ct, QS[kt]:QS[kt] + WIN],
                            start=True, stop=True,
                        )
                    nc.scalar.activation(
                        et[:, kt2:kt2 + 2, :], ps_s[:, :, :WIN], AF.Exp,
                        scale=SCALE,
                    )
                    eng = nc.vector
                    eng.tensor_tensor(
                        et[:, kt2:kt2 + 2, :],
                        et[:, kt2:kt2 + 2, :],
                        mask_sb[:, kt2:kt2 + 2, :],
                        OP.mult,
                    )

                for half in range(2):
                    ps_y = psA.tile([128, 2, 512], F32, tag="mm")
                    qb0 = half * 8
                    mms = []
                    for kt in range(NKT):
                        for qb in (kt, kt + 1, kt + 2):
                            if qb0 <= qb < qb0 + 8:
                                mms.append((kt, qb, divmod(qb - qb0, 4)[0]))
                    # start/stop per PSUM bank: each bank is its own
                    # accumulation group (start zero-marks only its bank)
                    first_i = {s: min(i for i, m in enumerate(mms) if m[2] == s)
                               for s in (0, 1)}
                    last_i = {s: max(i for i, m in enumerate(mms) if m[2] == s)
                              for s in (0, 1)}
                    for i, (kt, qb, sub) in enumerate(mms):
                        j0 = qb * 128 - QS[kt]
                        qo = (qb - qb0) % 4
                        nc.tensor.matmul(
                            ps_y[:, sub, qo * 128:(qo + 1) * 128],
                            v_sb[:, kt, h, :],
                            et[:, kt, j0:j0 + 128],
                            start=(i == first_i[sub]),
                            stop=(i == last_i[sub]),
                            skip_group_check=True,
                        )
                    rec = wk_pool.tile([HD, 2, 512], F32, tag="rec", bufs=2)
                    nc.vector.reciprocal_approx_fast(
                        rec[:, :, :], ps_y[0:HD, :, :]
                    )
                    nc.vector.tensor_tensor(
                        yn_sb[p0:p0 + HD, ct,
                              half * 1024:(half + 1) * 1024].rearrange(
                            "p (a b) -> p a b", a=2),
                        ps_y[HD:128, :, :],
                        rec[:, :, :],
                        OP.mult,
                    )

            # ---- partial output projection ----
            for qb in range(NKT):
                o_sb = wk_pool.tile([128, C], F32, tag="osb")
                ps = psA.tile([128, 2, 512], F32, tag="mm")
                for c in range(NM):
                    for j in range(2):
                        nc.tensor.matmul(
                            ps[:, j, :384],
                            yn_sb[:, c, qb * 128:(qb + 1) * 128],
                            wp_sb[:, c, j * 384:j * 384 + 384],
                            start=(c == 0),
                            stop=(c == NM - 1),
                            skip_group_check=True,
                        )
                src = ps[:, :, :384]
                dst = o_sb[:, :].rearrange("p (a b) -> p a b", a=2)
                if qb % 2 == 0:
                    nc.scalar.copy(dst, src)
                else:
                    nc.vector.tensor_copy(out=dst, in_=src)
                nc.sync.dma_start(out[qb * 128:(qb + 1) * 128, :], o_sb[:])

    nc.compile()
    return nc


_NC_CACHE = []


def _get_nc() -> bass.Bass:
    if not _NC_CACHE:
        _NC_CACHE.append(_build_nc())
    return _NC_CACHE[0]


def _make_mask() -> np.ndarray:
    p = np.arange(128)[:, None, None]
    kt = np.arange(NKT)[None, :, None]
    j = np.arange(WIN)[None, None, :]
    lk = 128 * kt + p
    qi = np.array(QS)[None, :, None] + j
    band = (qi >= lk) & (qi <= lk + WINDOW - 1)
    return band.astype(ml_dtypes.bfloat16).reshape(128, NKT * WIN)


def build_in_maps(x, W_attn, b_attn, W_proj, b_proj):
    x = np.asarray(x, dtype=np.float32)
    W_attn = np.asarray(W_attn, dtype=np.float32)
    b_attn = np.asarray(b_attn, dtype=np.float32)
    W_proj = np.asarray(W_proj, dtype=np.float32)

    bf = ml_dtypes.bfloat16
    mask_h = _make_mask()

    in_maps = []
    for core in range(8):
        b, g = divmod(core, 2)
        s = slice(g * GD, (g + 1) * GD)
        wq_h = W_attn[:, 0:C][:, s].astype(bf)
        wk_h = W_attn[:, C:2 * C][:, s].astype(bf)
        wv_h = W_attn[:, 2 * C:3 * C][:, s].astype(bf)
        wp_h = W_proj[s, :].astype(bf)
        bq_h = np.ascontiguousarray(
            b_attn[0:C][s].reshape(NM, 128).T).astype(np.float32)
        x_t = np.ascontiguousarray(x[b].T).astype(bf)
        in_maps.append({
            "x_t": x_t, "wq": wq_h, "wk": wk_h, "wv": wv_h, "wp": wp_h,
            "bq_t": bq_h, "mask": mask_h,
        })
    return in_maps


def kernel(x, W_attn, b_attn, W_proj, b_proj):
    in_maps = build_in_maps(x, W_attn, b_attn, W_proj, b_proj)
    nc = _get_nc()
    res = run_bass_kernel_spmd(nc, in_maps, list(range(8)))
    b_attn = np.asarray(b_attn, dtype=np.float32)
    W_proj = np.asarray(W_proj, dtype=np.float32)
    b_proj = np.asarray(b_proj, dtype=np.float32)
    out_bias = b_attn[2 * C:3 * C] @ W_proj + b_proj
    y = np.empty((B, T, C), dtype=np.float32)
    for b in range(B):
        y[b] = res.results[2 * b]["out"] + res.results[2 * b + 1]["out"] + out_bias
    return y
